# revision 1
# baseline (speedup 1.0000x reference)
"""ChebNet (K=3, 3 layers) GNN on 8 Trainium2 NeuronCores.

Math: per layer, out = h@(W0-W2) + L(h@W1 + 2*L(h@W2)) + b, where
L(v) = -dis * S(dis * v), S = unweighted scatter-add over edges, and
dis = rsqrt(clamp(outdeg,1)) masked by outdeg>0.  The per-edge weight
w = dis[src]*dis[dst] folds into two dense per-node row scalings.

Sharding: nodes split into 8 contiguous ranges (one per core, padded to
49*128 rows).  Each core owns the edges whose dst falls in its range and
computes output rows for its range only.  Before each graph op (lap) the
dis-scaled operand is AllGathered so every core can gather arbitrary src
rows with dma_gather.  dma_gather indices are int16, so the 50176-row
gathered tensor is addressed through two windows (rows [0,32768) and
[32768,50176)); each dst-tile's edges are split lo/hi by src window and
the two partial scatter sums merge for free in PSUM accumulation.

Scatter-add is done on the tensor engine: for each 128-edge chunk, a
[128e x 128dst] one-hot matrix is matmul'd against the gathered
[128e x 64f] rows, accumulating [128dst x 64f] in PSUM.  The one-hot
matrices depend only on the graph: they are built on the vector engine
once (first lap), stored to DRAM, and streamed back by DMA for the
remaining 5 laps.  dis tables come precomputed from the host.  Gather
calls rotate across 4 SWDGE queues (distinct Q7 core pairs + rings).
"""

import sys

sys.path.insert(0, "/opt/trn_rl_repo")

import numpy as np
from contextlib import ExitStack

_REAL = dict(N=50000, E=800000, NCORES=8, LO=32768, F0=128, F1=64, F2=16)


# ---------------------------------------------------------------- host prep
def _derive(cfg):
    c = dict(cfg)
    c["NPC"] = c["N"] // c["NCORES"]
    c["NT"] = -(-c["NPC"] // 128)
    c["NPAD"] = c["NT"] * 128
    c["NG"] = c["NCORES"] * c["NPAD"]
    c["FW"] = 64  # lap working width (256B gather elements)
    assert c["LO"] <= 32768 and c["NG"] - c["LO"] <= 32768
    assert c["N"] % c["NCORES"] == 0
    return c


def _prep(edge_index, c):
    N, E, NCORES, LO = c["N"], c["E"], c["NCORES"], c["LO"]
    NPC, NT, NPAD = c["NPC"], c["NT"], c["NPAD"]

    src = np.asarray(edge_index[0], dtype=np.int64)
    dst = np.asarray(edge_index[1], dtype=np.int64)
    assert src.shape == (E,) and dst.shape == (E,)
    psrc = (src // NPC) * NPAD + (src % NPC)  # padded global row of src

    cd = dst // NPC
    ld = dst - cd * NPC
    td = ld >> 7
    dl = ld & 127
    hi = (psrc >= LO).astype(np.int64)

    # ---- lap tables: edges grouped by (core, dst-tile, window), src-sorted
    counts = np.zeros((NCORES, NT, 2), np.int64)
    np.add.at(counts, (cd, td, hi), 1)
    Klo = np.maximum(1, -(-counts[:, :, 0].max(0) // 128))
    Khi = np.maximum(1, -(-counts[:, :, 1].max(0) // 128))
    LOFF = np.concatenate([[0], np.cumsum(Klo)]).astype(np.int64)
    HOFF = np.concatenate([[0], np.cumsum(Khi)]).astype(np.int64)
    TLO, THI = int(LOFF[-1]), int(HOFF[-1])

    order = np.lexsort((psrc, hi, td, cd))
    cd_s, td_s, hi_s = cd[order], td[order], hi[order]
    dl_s, psrc_s = dl[order], psrc[order]
    grp = (cd_s * NT + td_s) * 2 + hi_s
    gc = np.bincount(grp, minlength=NCORES * NT * 2)
    gstart = np.concatenate([[0], np.cumsum(gc)])[:-1]
    rank = np.arange(E) - gstart[grp]

    gidx_lo = np.zeros((NCORES, TLO * 128), np.int16)
    gidx_hi = np.zeros((NCORES, THI * 128), np.int16)
    dloc_lo = np.full((NCORES, 128, TLO), -1.0, np.float32)
    dloc_hi = np.full((NCORES, 128, THI), -1.0, np.float32)
    for cc in range(NCORES):
        for h, (gidx, dloc, OFF, base) in enumerate(
            [(gidx_lo, dloc_lo, LOFF, 0), (gidx_hi, dloc_hi, HOFF, LO)]
        ):
            m = (cd_s == cc) & (hi_s == h)
            slot = OFF[td_s[m]] + rank[m] // 128
            part = rank[m] & 127
            gidx[cc, slot * 128 + part] = (psrc_s[m] - base).astype(np.int16)
            dloc[cc, part, slot] = dl_s[m].astype(np.float32)

    # ---- dis tables from host-side degrees (replaces on-device degree pass)
    deg = np.bincount(src, minlength=N).astype(np.float64)
    dis_node = np.where(deg > 0, 1.0 / np.sqrt(np.maximum(deg, 1.0)), 0.0)
    dis_t = np.zeros((NCORES, 128, NT), np.float32)
    for cc in range(NCORES):
        dn = np.zeros(NPAD, np.float64)
        dn[:NPC] = dis_node[cc * NPC : (cc + 1) * NPC]
        dis_t[cc] = dn.reshape(NT, 128).T.astype(np.float32)

    def wrap(a):  # int16 [M*128] -> [128, M*8], idx j at [j%16, j//16], x8 replicated
        return np.tile(a.reshape(-1, 16).T, (8, 1)).copy()

    return dict(
        Klo=Klo, Khi=Khi, LOFF=LOFF, HOFF=HOFF, TLO=TLO, THI=THI,
        gidx_lo=[wrap(gidx_lo[cc]) for cc in range(NCORES)],
        gidx_hi=[wrap(gidx_hi[cc]) for cc in range(NCORES)],
        dloc_lo=dloc_lo, dloc_hi=dloc_hi,
        dis=dis_t, negdis=-dis_t, n2dis2=(-2.0 * dis_t * dis_t).astype(np.float32),
    )


# ---------------------------------------------------------------- device build
def _build(c, pp, Fins, use_bias):
    import concourse.bacc as bacc
    import concourse.tile as tile
    from concourse import mybir

    f32, i16 = mybir.dt.float32, mybir.dt.int16
    bf16 = mybir.dt.bfloat16
    AOT = mybir.AluOpType
    NT, NPAD, NG, LO, FW = c["NT"], c["NPAD"], c["NG"], c["LO"], c["FW"]
    NCORES, F0, F2 = c["NCORES"], c["F0"], c["F2"]
    TLO, THI = pp["TLO"], pp["THI"]
    Klo, Khi = pp["Klo"], pp["Khi"]
    LOFF, HOFF = pp["LOFF"], pp["HOFF"]
    KMAX = int(max(Klo.max(), Khi.max()))
    NQ = 4  # SWDGE queues to rotate gather calls over
    GRP = 4
    groups = [list(range(g * GRP, min((g + 1) * GRP, NT))) for g in range(-(-NT // GRP))]

    nc = bacc.Bacc(num_devices=NCORES, num_swdge_queues=NQ)

    xin = nc.dram_tensor("x", [NPAD, F0], f32, kind="ExternalInput")
    gl_d = nc.dram_tensor("gidx_lo", [128, TLO * 8], i16, kind="ExternalInput")
    gh_d = nc.dram_tensor("gidx_hi", [128, THI * 8], i16, kind="ExternalInput")
    dl_d = nc.dram_tensor("dloc_lo", [128, TLO], f32, kind="ExternalInput")
    dh_d = nc.dram_tensor("dloc_hi", [128, THI], f32, kind="ExternalInput")
    dis_d = nc.dram_tensor("dis", [128, NT], f32, kind="ExternalInput")
    ndis_d = nc.dram_tensor("negdis", [128, NT], f32, kind="ExternalInput")
    n2d2_d = nc.dram_tensor("n2dis2", [128, NT], f32, kind="ExternalInput")
    iota_d = nc.dram_tensor("iota", [128, 128], f32, kind="ExternalInput")
    id_d = nc.dram_tensor("ident", [128, 128], f32, kind="ExternalInput")
    W_d = {}
    for l in range(3):
        for nm in ("wa", "wb", "wc"):
            W_d[nm, l] = nc.dram_tensor(f"{nm}{l}", [Fins[l], FW], f32, kind="ExternalInput")
        if use_bias[l]:
            W_d["br", l] = nc.dram_tensor(f"br{l}", [128, FW], f32, kind="ExternalInput")
    y_d = nc.dram_tensor("y", [NPAD, F2], f32, kind="ExternalOutput")

    ag_in = [nc.dram_tensor(f"agin{i}", [NPAD, FW], f32) for i in range(6)]
    ag_out = [nc.dram_tensor(f"agout{i}", [NG, FW], f32, addr_space="Shared") for i in range(6)]
    oh_lo_d = nc.dram_tensor("ohlo", [128, TLO * 128], bf16)
    oh_hi_d = nc.dram_tensor("ohhi", [128, THI * 128], bf16)

    xv = xin.rearrange("(t p) f -> p t f", p=128)
    yv = y_d.rearrange("(t p) f -> p t f", p=128)

    with tile.TileContext(nc) as tc, ExitStack() as ctx:
        cst = ctx.enter_context(tc.tile_pool(name="cst", bufs=1))
        big = ctx.enter_context(tc.tile_pool(name="big", bufs=1))
        gp = ctx.enter_context(tc.tile_pool(name="gp", bufs=2))
        gpb = ctx.enter_context(tc.tile_pool(name="gpb", bufs=3))
        ohp = ctx.enter_context(tc.tile_pool(name="ohp", bufs=2))
        smp = ctx.enter_context(tc.tile_pool(name="smp", bufs=4))
        slb = ctx.enter_context(tc.tile_pool(name="slb", bufs=2))
        psA = ctx.enter_context(tc.tile_pool(name="psA", bufs=2, space="PSUM"))
        psT = ctx.enter_context(tc.tile_pool(name="psT", bufs=2, space="PSUM"))
        psD = ctx.enter_context(tc.tile_pool(name="psD", bufs=2, space="PSUM"))

        # constants (f32 masters staged through recycled tiles, kept as bf16)
        iota_f = smp.tile([128, 128], f32, tag="stage")
        nc.sync.dma_start(iota_f[:], iota_d[:])
        ident_f = smp.tile([128, 128], f32, tag="stage")
        nc.sync.dma_start(ident_f[:], id_d[:])
        gl = cst.tile([128, TLO * 8], i16)
        nc.sync.dma_start(gl[:], gl_d[:])
        gh = cst.tile([128, THI * 8], i16)
        nc.sync.dma_start(gh[:], gh_d[:])
        dlo = cst.tile([128, TLO], f32)
        nc.sync.dma_start(dlo[:], dl_d[:])
        dhi = cst.tile([128, THI], f32)
        nc.sync.dma_start(dhi[:], dh_d[:])
        dis = cst.tile([128, NT], f32)
        nc.sync.dma_start(dis[:], dis_d[:])
        negdis = cst.tile([128, NT], f32)
        nc.sync.dma_start(negdis[:], ndis_d[:])
        n2dis2 = cst.tile([128, NT], f32)
        nc.sync.dma_start(n2dis2[:], n2d2_d[:])
        Wt = {}
        Wb = {}
        for k, d in W_d.items():
            if k[0] == "br":
                Wt[k] = cst.tile([128, FW], f32, name=f"w_{k[0]}_{k[1]}", tag=f"w_{k[0]}_{k[1]}")
                nc.sync.dma_start(Wt[k][: d.shape[0], :], d[:])
            else:
                wstage = smp.tile([128, FW], f32, name=f"ws_{k[0]}_{k[1]}", tag="wstage")
                nc.sync.dma_start(wstage[: d.shape[0], :], d[:])
                Wb[k] = cst.tile([128, FW], bf16, name=f"wb_{k[0]}_{k[1]}", tag=f"wb_{k[0]}_{k[1]}")
                nc.scalar.copy(Wb[k][: d.shape[0], :], wstage[: d.shape[0], :])
        ident_b = cst.tile([128, 128], bf16)
        nc.scalar.copy(ident_b[:], ident_f[:])
        iota_b = cst.tile([128, 128], bf16)
        nc.scalar.copy(iota_b[:], iota_f[:])

        # ---------------- lap helper
        qctr = [0]

        def lap(agi, epi):
            """Gather rows of ag_out[agi] per edge, scatter-add per dst tile,
            call epi(t, acc_psum) with the [128,FW] PSUM partial sums.
            agi==0 builds the one-hot slabs on DVE and stores them to DRAM;
            agi>0 streams them back instead."""
            build = agi == 0
            src_lo = ag_out[agi][0:LO, :]
            src_hi = ag_out[agi][LO:NG, :]
            for tl in groups:
                a_lo, b_lo = int(LOFF[tl[0]]), int(LOFF[tl[-1] + 1])
                a_hi, b_hi = int(HOFF[tl[0]]), int(HOFF[tl[-1] + 1])
                nlo, nhi = b_lo - a_lo, b_hi - a_hi
                CAP = 8  # max 128-chunks (1024 idx) per dma_gather call
                glo = gp.tile([128, nlo, FW], f32, tag="glo")
                for o in range(0, nlo, CAP):
                    n = min(CAP, nlo - o)
                    nc.gpsimd.dma_gather(
                        glo[:, o : o + n, :], src_lo,
                        gl[:, (a_lo + o) * 8 : (a_lo + o + n) * 8],
                        num_idxs=n * 128, num_idxs_reg=n * 128, elem_size=FW,
                        queue_num=qctr[0] % NQ,
                    )
                    qctr[0] += 1
                ghi_t = gp.tile([128, nhi, FW], f32, tag="ghi")
                for o in range(0, nhi, CAP):
                    n = min(CAP, nhi - o)
                    nc.gpsimd.dma_gather(
                        ghi_t[:, o : o + n, :], src_hi,
                        gh[:, (a_hi + o) * 8 : (a_hi + o + n) * 8],
                        num_idxs=n * 128, num_idxs_reg=n * 128, elem_size=FW,
                        queue_num=qctr[0] % NQ,
                    )
                    qctr[0] += 1
                glo_b = gpb.tile([128, nlo, FW], bf16, tag="glob")
                nc.scalar.copy(glo_b[:], glo[:])
                ghi_b = gpb.tile([128, nhi, FW], bf16, tag="ghib")
                nc.scalar.copy(ghi_b[:], ghi_t[:])
                for t in tl:
                    klo, khi = int(Klo[t]), int(Khi[t])
                    slab_lo = ohp.tile([128, KMAX * 128], bf16, tag="slab_lo")
                    slab_hi = ohp.tile([128, KMAX * 128], bf16, tag="slab_hi")
                    if build:
                        for k in range(klo):
                            s = int(LOFF[t]) + k
                            nc.vector.tensor_scalar(
                                slab_lo[:, k * 128 : (k + 1) * 128],
                                iota_b[:], dlo[:, s : s + 1], None, AOT.is_equal,
                            )
                        for k in range(khi):
                            s = int(HOFF[t]) + k
                            nc.vector.tensor_scalar(
                                slab_hi[:, k * 128 : (k + 1) * 128],
                                iota_b[:], dhi[:, s : s + 1], None, AOT.is_equal,
                            )
                        nc.sync.dma_start(
                            oh_lo_d[:, int(LOFF[t]) * 128 : (int(LOFF[t]) + klo) * 128],
                            slab_lo[:, : klo * 128],
                        )
                        nc.sync.dma_start(
                            oh_hi_d[:, int(HOFF[t]) * 128 : (int(HOFF[t]) + khi) * 128],
                            slab_hi[:, : khi * 128],
                        )
                    else:
                        nc.sync.dma_start(
                            slab_lo[:, : klo * 128],
                            oh_lo_d[:, int(LOFF[t]) * 128 : (int(LOFF[t]) + klo) * 128],
                        )
                        nc.sync.dma_start(
                            slab_hi[:, : khi * 128],
                            oh_hi_d[:, int(HOFF[t]) * 128 : (int(HOFF[t]) + khi) * 128],
                        )
                    acc = psA.tile([128, FW], f32, tag="acc")
                    ntot = klo + khi
                    i = 0
                    for k in range(klo):
                        s = int(LOFF[t]) + k
                        nc.tensor.matmul(
                            acc[:], slab_lo[:, k * 128 : (k + 1) * 128],
                            glo_b[:, s - a_lo, :],
                            start=(i == 0), stop=(i == ntot - 1),
                        )
                        i += 1
                    for k in range(khi):
                        s = int(HOFF[t]) + k
                        nc.tensor.matmul(
                            acc[:], slab_hi[:, k * 128 : (k + 1) * 128],
                            ghi_b[:, s - a_hi, :],
                            start=(i == 0), stop=(i == ntot - 1),
                        )
                        i += 1
                    epi(t, acc)

        # ---------------- layers
        HALF = (NT // 2) * 128  # sub-AllGather split point (rows)

        def sub_ag(agi, buf):
            nc.sync.dma_start(ag_in[agi].rearrange("(t p) f -> p t f", p=128), buf[:])
            nc.gpsimd.collective_compute(
                "AllGather", mybir.AluOpType.bypass,
                replica_groups=[list(range(NCORES))],
                ins=[ag_in[agi][:, :]], outs=[ag_out[agi][:, :]],
            )

        h_prev = None
        for l in range(3):
            Fin = Fins[l]
            As = big.tile([128, NT, FW], f32, tag="As")
            Cs1 = big.tile([128, NT, FW], f32, tag="Cs1")
            Oa = big.tile([128, NT, FW], f32, tag="Oa")
            hT_all = big.tile([128, NT * 128], bf16, tag="hTa")
            hsT_all = big.tile([128, NT * 128], bf16, tag="hsTa")
            # pass 1: transposes + As (the AllGather input) only
            for tl in groups:
                w = len(tl) * 128
                c0 = tl[0] * 128
                for u, t in enumerate(tl):
                    if l == 0:
                        ht = smp.tile([128, F0], f32, tag="xt")
                        nc.sync.dma_start(ht[:], xv[:, t, :])
                        ht_b = smp.tile([128, F0], bf16, tag="xtb")
                        nc.scalar.copy(ht_b[:], ht[:])
                        ht_ap = ht_b[:]
                    else:
                        ht_ap = h_prev[:, t, :]
                    ps = psT.tile([128, 128], bf16, tag="pt")
                    nc.tensor.transpose(ps[:Fin, :], ht_ap, ident_b[:])
                    nc.scalar.copy(hT_all[:Fin, (t * 128) : (t + 1) * 128], ps[:Fin, :])
                    hs = smp.tile([128, Fin], bf16, tag="hs")
                    nc.vector.tensor_scalar_mul(hs[:], ht_ap, dis[:, t : t + 1])
                    ps2 = psT.tile([128, 128], bf16, tag="pt")
                    nc.tensor.transpose(ps2[:Fin, :], hs[:], ident_b[:])
                    nc.scalar.copy(hsT_all[:Fin, (t * 128) : (t + 1) * 128], ps2[:Fin, :])
                pd = psD.tile([64, GRP * 128], f32, tag="pd")
                nc.tensor.matmul(pd[:, :w], Wb["wc", l][:Fin, :], hsT_all[:Fin, c0 : c0 + w])
                pT = slb.tile([64, GRP * 128], bf16, tag="pT")
                nc.scalar.copy(pT[:, :w], pd[:, :w])
                for u, t in enumerate(tl):
                    pb = psT.tile([128, 128], bf16, tag="pt")
                    nc.tensor.transpose(
                        pb[:, :FW], pT[:FW, u * 128 : (u + 1) * 128], ident_b[:FW, :FW]
                    )
                    nc.scalar.copy(As[:, t, :], pb[:, :FW])

            agA = 2 * l
            sub_ag(agA, As)

            # pass 2: Cs1 and Oa — overlaps the first lap's gathers
            for tl in groups:
                w = len(tl) * 128
                c0 = tl[0] * 128
                for dstbuf, wkey, srcT in (
                    (Cs1, ("wb", l), hsT_all),
                    (Oa, ("wa", l), hT_all),
                ):
                    pd = psD.tile([64, GRP * 128], f32, tag="pd")
                    nc.tensor.matmul(pd[:, :w], Wb[wkey][:Fin, :], srcT[:Fin, c0 : c0 + w])
                    pT = slb.tile([64, GRP * 128], bf16, tag="pT")
                    nc.scalar.copy(pT[:, :w], pd[:, :w])
                    for u, t in enumerate(tl):
                        pb = psT.tile([128, 128], bf16, tag="pt")
                        nc.tensor.transpose(
                            pb[:, :FW], pT[:FW, u * 128 : (u + 1) * 128], ident_b[:FW, :FW]
                        )
                        nc.scalar.copy(dstbuf[:, t, :], pb[:, :FW])

            Cs = big.tile([128, NT, FW], f32, tag="Cs")

            def epi1(t, acc):
                tmp = smp.tile([128, FW], f32, tag="t1")
                nc.vector.tensor_scalar_mul(tmp[:], acc[:], n2dis2[:, t : t + 1])
                nc.vector.tensor_add(Cs[:, t, :], Cs1[:, t, :], tmp[:])

            lap(agA, epi1)

            agC = 2 * l + 1
            sub_ag(agC, Cs)

            hn = big.tile([128, NT, FW], bf16 if l < 2 else f32, tag=f"h{l}")

            def epi2(t, acc):
                tmp = smp.tile([128, FW], f32, tag="t1")
                nc.vector.tensor_scalar_mul(tmp[:], acc[:], negdis[:, t : t + 1])
                if use_bias[l]:
                    tmp2 = smp.tile([128, FW], f32, tag="t2")
                    nc.vector.tensor_add(tmp2[:], tmp[:], Oa[:, t, :])
                    pre = smp.tile([128, FW], f32, tag="t3")
                    nc.vector.tensor_add(pre[:], tmp2[:], Wt["br", l][:, :])
                else:
                    pre = smp.tile([128, FW], f32, tag="t2")
                    nc.vector.tensor_add(pre[:], tmp[:], Oa[:, t, :])
                if l < 2:
                    nc.vector.tensor_scalar_max(hn[:, t, :], pre[:], 0.0)
                else:
                    nc.vector.tensor_copy(hn[:, t, :], pre[:])

            lap(agC, epi2)
            h_prev = hn

        nc.sync.dma_start(yv[:], h_prev[:, :, :F2])

    nc.compile()
    return nc


# ---------------------------------------------------------------- entry
def _run(x, edge_index, Ws, bs, cfg=None, trace=False):
    from concourse.bass_utils import run_bass_kernel_spmd

    c = _derive(cfg or _REAL)
    N, NCORES, NPC, NPAD = c["N"], c["NCORES"], c["NPC"], c["NPAD"]
    F0, F2, FW = c["F0"], c["F2"], c["FW"]

    x = np.ascontiguousarray(np.asarray(x, dtype=np.float32))
    pp = _prep(edge_index, c)

    Fins = [F0, c["F1"], c["F1"]]
    use_bias = [bool(np.any(b)) for b in bs]
    nc = _build(c, pp, Fins, use_bias)

    iota = np.tile(np.arange(128, dtype=np.float32), (128, 1))
    ident = np.eye(128, dtype=np.float32)

    def padW(w, fin):
        out = np.zeros((fin, FW), np.float32)
        out[: w.shape[0], : w.shape[1]] = w
        return out

    base = {"iota": iota, "ident": ident}
    for l in range(3):
        W = np.asarray(Ws[l], dtype=np.float32)
        base[f"wa{l}"] = padW(W[0] - W[2], Fins[l])
        base[f"wb{l}"] = padW(W[1], Fins[l])
        base[f"wc{l}"] = padW(W[2], Fins[l])
        if use_bias[l]:
            br = np.zeros((128, FW), np.float32)
            br[:, : bs[l].shape[0]] = np.asarray(bs[l], np.float32)
            base[f"br{l}"] = br

    in_maps = []
    for cc in range(NCORES):
        xl = np.zeros((NPAD, F0), np.float32)
        xl[:NPC] = x[cc * NPC : (cc + 1) * NPC]
        in_maps.append(
            dict(
                base,
                x=xl,
                gidx_lo=pp["gidx_lo"][cc],
                gidx_hi=pp["gidx_hi"][cc],
                dloc_lo=np.ascontiguousarray(pp["dloc_lo"][cc]),
                dloc_hi=np.ascontiguousarray(pp["dloc_hi"][cc]),
                dis=np.ascontiguousarray(pp["dis"][cc]),
                negdis=np.ascontiguousarray(pp["negdis"][cc]),
                n2dis2=np.ascontiguousarray(pp["n2dis2"][cc]),
            )
        )

    res = run_bass_kernel_spmd(nc, in_maps, core_ids=list(range(NCORES)), trace=trace)
    out = np.concatenate([res.results[cc]["y"][:NPC] for cc in range(NCORES)], axis=0)
    return out[:, :F2], res


def kernel(x, edge_index, W1, b1, Wm, bm, W2, b2):
    out, _ = _run(
        np.asarray(x), np.asarray(edge_index),
        [np.asarray(W1), np.asarray(Wm), np.asarray(W2)],
        [np.asarray(b1), np.asarray(bm), np.asarray(b2)],
    )
    return out

